# revision 35
# baseline (speedup 1.0000x reference)
"""Trainium2 Bass kernel for nn_BasicCNN (conv bank + LoRA-masked recurrent net).

Pure data-parallel over batch (128 rows/core on 8 cores), ZERO collectives —
under the conservative collective cost model (15us + bytes/40GBps) any
TP-style per-timestep AllGather dwarfs the actual compute, so each core runs
the full network on its batch shard instead:

 - W1 = W + 2*(A@B)*mask + I is built on HOST (fp32) and shipped bf16.
   The +I fold implements the residual. Rows 0:KT_RES*128 are SBUF-resident:
   rows 0:SEN stream INTO the resident tile in t1-consumption order (t1's
   stream IS the resident load, nothing shipped twice); the remaining k-slabs
   are streamed per timestep as [128, 1024] chunks (one per (m-group, k))
   round-robin over the three DMA queues (SP/Pool/Act), ring-buffered so DMA
   hides under the PE matmuls.
 - State kept transposed k-major ([state_dim, batch] in 32 slabs of
   [128, 128]) so W tiles are the stationary operand and the matmul output
   [m-part, batch] is directly the next state slab — no transposes ever.
 - PSUM allows only 8 bank-aligned accumulators, so each timestep runs 4
   m-groups x 8 banks; all relu/bias drains live on DVE (plus Act for the
   tail copies) so no DMA queue mixes in compute.
 - t1 contracts only the sensory block (state1 is zero past SEN); t4 runs as
   two 4-bank column halves with out-proj group 0's contraction split so PE
   never waits on a full drain barrier; out-proj stationary is SBUF-resident
   (loaded during t2 slack); output bias is added on the host.
 - conv bank = one dense [512, 3328] matmul vs a host-assembled scatter of
   the conv kernels, loaded as 16 column-chunks so conv starts ~2.5us in.
"""
import sys

for _p in ("/opt/trn_rl_repo", "/root/.axon_site/_ro/trn_rl_repo"):
    if _p not in sys.path:
        sys.path.append(_p)

import numpy as np
import ml_dtypes

import concourse.bacc as bacc
import concourse.mybir as mybir
import concourse.tile as tile
from concourse.bass_utils import run_bass_kernel_spmd

dt = mybir.dt
BF16 = ml_dtypes.bfloat16
AF = mybir.ActivationFunctionType

N_CORES = 8
B = 1024
HW = 8
C_IN = 8
FN = 16
SEN, INT, OUT = 1024, 2048, 1024
TOT = 4096
CNN_OUT = 3264
CNN_PAD = 3328
NUM_OUT = 1968
NUM_PAD = 2048
LORA_SCALE = 2.0

BSH = B // N_CORES           # 128 batch rows per core
KT = TOT // 128              # 32 k-slabs of state/W
KT_RES = 10                  # W1 k-slabs resident in SBUF
KT_STR = KT - KT_RES         # 19 streamed per full timestep
NG = 4                       # m-groups per timestep
GM = 8                       # m-tiles per group (= PSUM banks)
CONV_MT = CNN_PAD // 128     # 26
SEN_KT = SEN // 128          # 8
OUT_KT = OUT // 128          # 8 (O-block k-slabs for out proj)
OMT = NUM_PAD // 128         # 16


def _build_program(reps: int = 1, use_cc: bool = True):
    nc = bacc.Bacc("TRN2", target_bir_lowering=False, debug=False,
                   enable_asserts=True, num_devices=N_CORES)

    xT_d = nc.dram_tensor("xT", [512, BSH], dt.bfloat16, kind="ExternalInput")
    wbig_d = nc.dram_tensor("wbig", [512, CNN_PAD], dt.bfloat16, kind="ExternalInput")
    cbias_d = nc.dram_tensor("cbias", [CNN_PAD], dt.float32, kind="ExternalInput")
    ipw_d = nc.dram_tensor("ipw", [CNN_PAD, SEN], dt.bfloat16, kind="ExternalInput")
    ipb_d = nc.dram_tensor("ipb", [SEN], dt.float32, kind="ExternalInput")
    w1r_d = nc.dram_tensor("w1r", [KT_RES * 128, TOT], dt.bfloat16, kind="ExternalInput")
    w1s_d = nc.dram_tensor("w1s", [KT_STR * 128, TOT], dt.bfloat16, kind="ExternalInput")
    oww_d = nc.dram_tensor("oww", [OUT, NUM_PAD], dt.bfloat16, kind="ExternalInput")

    # partition-major output layout [p, m, b]; host maps row j = m*128+p
    outT_d = nc.dram_tensor("outT", [128, OMT, BSH], dt.float32, kind="ExternalOutput")

    with tile.TileContext(nc) as tc:
        with tc.tile_pool(name="pers", bufs=1) as pers, \
             tc.tile_pool(name="psum", bufs=8, space="PSUM") as psp, \
             tc.tile_pool(name="wbigp", bufs=4) as wbp, \
             tc.tile_pool(name="ipwp", bufs=10) as ipp, \
             tc.tile_pool(name="wchk", bufs=8) as wcp, \
             tc.tile_pool(name="outp", bufs=1) as otp:

            wres = pers.tile([128, KT_RES, TOT], dt.bfloat16, tag="wres")
            st_a = pers.tile([128, KT, BSH], dt.bfloat16, tag="st_a")
            st_b = pers.tile([128, KT, BSH], dt.bfloat16, tag="st_b")
            ostate = pers.tile([128, OUT_KT, BSH], dt.bfloat16, tag="ostate")
            featT = pers.tile([128, CONV_MT, BSH], dt.bfloat16, tag="featT")
            oww_sb = pers.tile([128, OUT_KT, NUM_PAD], dt.bfloat16, tag="oww_sb")
            xT_sb = pers.tile([128, 4, BSH], dt.bfloat16, tag="xT_sb")
            cbias_sb = pers.tile([128, CONV_MT], dt.float32, tag="cbias_sb")
            ipb_sb = pers.tile([128, SEN_KT], dt.float32, tag="ipb_sb")

            # unified round-robin over the three DMA-capable queues; all
            # elementwise drains live on DVE so no queue mixes DMA + compute
            dmaq = [nc.gpsimd, nc.sync, nc.scalar]
            qctr = [0]

            def dma(out, in_):
                dmaq[qctr[0] % 3].dma_start(out=out, in_=in_)
                qctr[0] += 1

            warm_in = pers.tile([128, 1], dt.float32, tag="warm_in")
            warm_out = pers.tile([128, 1], dt.float32, tag="warm_out")
            nc.gpsimd.memset(warm_in[:], 0.0)

            nc.gpsimd.dma_start(out=xT_sb[:, :, :],
                                in_=xT_d.rearrange("(k p) b -> p k b", p=128))
            nc.sync.dma_start(out=cbias_sb[:], in_=cbias_d.rearrange("(m p) -> p m", p=128))
            nc.sync.dma_start(out=ipb_sb[:], in_=ipb_d.rearrange("(m p) -> p m", p=128))

            for rep in range(reps):
                # ---- conv bank: one dense matmul vs scattered conv kernels ----
                # wbig loaded as 16 column-chunks round-robin over all three
                # DMA queues so conv's first m-tiles start ~2.5us in
                wbig_t = [wbp.tile([128, CNN_PAD], dt.bfloat16, tag="wbig",
                                   name=f"wbig{rep}_{i}") for i in range(4)]
                col_waves = [(0, 256), (256, 832), (832, 1664),
                             (1664, 2496), (2496, CNN_PAD)]
                for c0, c1 in col_waves:
                    for kk in range(4):
                        dma(wbig_t[kk][:, c0:c1], wbig_d[kk * 128:(kk + 1) * 128, c0:c1])
                for m in range(CONV_MT):
                    ps = psp.tile([128, BSH], dt.float32, tag="ps", name=f"cv{rep}_{m}")
                    for kk in range(4):
                        nc.tensor.matmul(ps[:], wbig_t[kk][:, m * 128:(m + 1) * 128],
                                         xT_sb[:, kk, :], start=(kk == 0), stop=(kk == 3))
                    nc.vector.tensor_scalar(featT[:, m, :], ps[:], cbias_sb[:, m:m + 1],
                                            0.0, op0=mybir.AluOpType.add,
                                            op1=mybir.AluOpType.max)

                # ---- input proj -> state1 (k-outer over 26 ipw slabs) ----
                ip_ps = [psp.tile([128, BSH], dt.float32, tag="ps", name=f"ip{rep}_{m}")
                         for m in range(SEN_KT)]
                for k in range(CONV_MT):
                    t = ipp.tile([128, SEN], dt.bfloat16, tag="ipw")
                    dma(t[:], ipw_d[k * 128:(k + 1) * 128, :])
                    for m in range(SEN_KT):
                        nc.tensor.matmul(ip_ps[m][:], t[:, m * 128:(m + 1) * 128],
                                         featT[:, k, :], start=(k == 0),
                                         stop=(k == CONV_MT - 1))
                for m in range(SEN_KT):
                    nc.vector.tensor_scalar(st_a[:, m, :], ip_ps[m][:],
                                            ipb_sb[:, m:m + 1], 0.0,
                                            op0=mybir.AluOpType.add,
                                            op1=mybir.AluOpType.max)

                # ---- t1 (contracts only the SEN block, k0..8): the
                # [128, 1024] pieces stream straight INTO the resident wres
                # tile in consumption order — t1's stream IS the resident
                # load for rows 0:SEN, nothing is shipped twice ----
                cur, nxt = st_a, st_b
                for mg in range(NG):
                    ps = [psp.tile([128, BSH], dt.float32, tag="ps",
                                   name=f"t1r{rep}g{mg}_{i}") for i in range(GM)]
                    for k in range(SEN_KT):
                        if rep == 0:
                            # hold these off the queues until ~14us so they
                            # cannot displace the ip-phase ipw stream; t1
                            # first reads them at ~19us
                            with tc.tile_wait_until(0.014):
                                dma(wres[:, k, mg * 1024:(mg + 1) * 1024],
                                    w1r_d[k * 128:(k + 1) * 128,
                                          mg * 1024:(mg + 1) * 1024])
                        for m8 in range(GM):
                            m = mg * GM + m8
                            nc.tensor.matmul(ps[m8][:],
                                             wres[:, k, m * 128:(m + 1) * 128],
                                             cur[:, k, :],
                                             start=(k == 0), stop=(k == SEN_KT - 1))
                    for m8 in range(GM):
                        nc.vector.tensor_scalar_max(nxt[:, mg * GM + m8, :],
                                                    ps[m8][:], 0.0)
                    if rep == 0:
                        # the two k-slabs past SEN land behind each group
                        with tc.tile_wait_until(0.021):
                            for kr in range(SEN_KT, KT_RES):
                                dma(wres[:, kr, mg * 1024:(mg + 1) * 1024],
                                    w1r_d[kr * 128:(kr + 1) * 128,
                                          mg * 1024:(mg + 1) * 1024])
                cur, nxt = nxt, cur

                # warm the Act engine's activation table (Copy) off the
                # critical path so the tail bias-adds pay no table load
                if rep == 0:
                    nc.scalar.activation(warm_out[:], warm_in[:], AF.Copy, bias=0.0)

                # ---- recurrence t2..t3 (full contraction) ----
                for t in (2, 3):
                    for mg in range(NG):
                        if rep == 0 and t == 2:
                            # out-proj stationary resident load, slipped into
                            # t2's DMA slack (2 k-slabs per group)
                            dma(oww_sb[:, 2 * mg:2 * mg + 2, :],
                                oww_d[256 * mg:256 * (mg + 1), :]
                                .rearrange("(k p) o -> p k o", p=128))
                        ps = [psp.tile([128, BSH], dt.float32, tag="ps",
                                       name=f"t{t}r{rep}g{mg}_{i}") for i in range(GM)]
                        chunk = None
                        for k in range(KT):
                            if k >= KT_RES:
                                chunk = wcp.tile([128, 1024], dt.bfloat16, tag="wchk")
                                dma(chunk[:],
                                    w1s_d[(k - KT_RES) * 128:(k - KT_RES + 1) * 128,
                                          mg * 1024:(mg + 1) * 1024])
                            for m8 in range(GM):
                                m = mg * GM + m8
                                if k < KT_RES:
                                    wap = wres[:, k, m * 128:(m + 1) * 128]
                                else:
                                    wap = chunk[:, m8 * 128:(m8 + 1) * 128]
                                nc.tensor.matmul(ps[m8][:], wap, cur[:, k, :],
                                                 start=(k == 0), stop=(k == KT - 1))
                        for m8 in range(GM):
                            nc.vector.tensor_scalar_max(nxt[:, mg * GM + m8, :],
                                                        ps[m8][:], 0.0)
                    cur, nxt = nxt, cur

                # ---- t4 (O block only) in two 4-bank column halves, then
                # out-proj in four 4-bank groups with the first group's
                # contraction split so PE never stalls on a full drain
                # barrier; output bias is added on the host ----
                ostg = otp.tile([128, OMT, BSH], dt.float32, tag="ostg")

                t4_ps = {}
                for half in range(2):
                    for m4 in range(4):
                        t4_ps[(half, m4)] = psp.tile(
                            [128, BSH], dt.float32, tag="ps",
                            name=f"t4r{rep}h{half}_{m4}")
                    if half == 0:
                        op_g0 = [psp.tile([128, BSH], dt.float32, tag="ps",
                                          name=f"opr{rep}g0_{i}") for i in range(4)]
                    chunk = None
                    for k in range(KT):
                        if k >= KT_RES:
                            chunk = wcp.tile([128, 512], dt.bfloat16, tag="wchk",
                                             name=f"t4ch{rep}h{half}_{k}")
                            dma(chunk[:],
                                w1s_d[(k - KT_RES) * 128:(k - KT_RES + 1) * 128,
                                      3072 + half * 512:3072 + (half + 1) * 512])
                        for m4 in range(4):
                            c0 = 3072 + half * 512 + m4 * 128
                            if k < KT_RES:
                                wap = wres[:, k, c0:c0 + 128]
                            else:
                                wap = chunk[:, m4 * 128:(m4 + 1) * 128]
                            nc.tensor.matmul(t4_ps[(half, m4)][:], wap, cur[:, k, :],
                                             start=(k == 0), stop=(k == KT - 1))
                    for m4 in range(4):
                        sl = half * 4 + m4
                        if m4 % 2 == 0:
                            nc.vector.tensor_scalar_max(ostate[:, sl, :],
                                                        t4_ps[(half, m4)][:], 0.0)
                        else:
                            nc.scalar.activation(ostate[:, sl, :],
                                                 t4_ps[(half, m4)][:], AF.Relu)
                    if half == 0:
                        # partial out-proj group 0 over the first four ostate
                        # slabs while t4's second half still computes
                        for k in range(4):
                            for j4 in range(4):
                                nc.tensor.matmul(op_g0[j4][:],
                                                 oww_sb[:, k, j4 * 128:(j4 + 1) * 128],
                                                 ostate[:, k, :],
                                                 start=(k == 0), stop=False)

                def opj_drain(g, ps4):
                    for j4 in range(4):
                        jm = g * 4 + j4
                        if j4 % 2 == 0:
                            nc.vector.tensor_scalar_add(ostg[:, jm, :], ps4[j4][:], 0.0)
                        else:
                            nc.scalar.activation(ostg[:, jm, :], ps4[j4][:],
                                                 AF.Copy, bias=0.0)
                        if g == 3 and j4 == 1:
                            dma(outT_d[:, 12:14, :], ostg[:, 12:14, :])
                    if g == 3:
                        dma(outT_d[:, 14:16, :], ostg[:, 14:16, :])
                    else:
                        dma(outT_d[:, g * 4:(g + 1) * 4, :],
                            ostg[:, g * 4:(g + 1) * 4, :])

                # finish group 0 (k4..8) and run groups 1..3
                for k in range(4, OUT_KT):
                    for j4 in range(4):
                        nc.tensor.matmul(op_g0[j4][:],
                                         oww_sb[:, k, j4 * 128:(j4 + 1) * 128],
                                         ostate[:, k, :], start=False,
                                         stop=(k == OUT_KT - 1))
                opj_drain(0, op_g0)
                for g in range(1, 4):
                    psg = [psp.tile([128, BSH], dt.float32, tag="ps",
                                    name=f"opr{rep}g{g}_{i}") for i in range(4)]
                    for k in range(OUT_KT):
                        for j4 in range(4):
                            nc.tensor.matmul(psg[j4][:],
                                             oww_sb[:, k,
                                                    g * 512 + j4 * 128:g * 512 + (j4 + 1) * 128],
                                             ostate[:, k, :], start=(k == 0),
                                             stop=(k == OUT_KT - 1))
                    opj_drain(g, psg)

    nc.compile()
    return nc


_PROGRAM_CACHE: dict = {}


def get_program(reps: int = 1, use_cc: bool = True):
    key = (reps, use_cc)
    if key not in _PROGRAM_CACHE:
        _PROGRAM_CACHE[key] = _build_program(reps, use_cc)
    return _PROGRAM_CACHE[key]


def _assemble_wbig(inputs):
    wbig = np.zeros((512, CNN_PAD), np.float32)
    cbias = np.zeros(CNN_PAD, np.float32)
    off = 0
    for k in range(1, 9):
        o = HW - k + 1
        w = np.asarray(inputs[f"conv_w{k}"], np.float32)
        cb = np.asarray(inputs["conv_b"], np.float32)[k - 1]
        py = np.arange(o)[:, None, None]
        px = np.arange(o)[None, :, None]
        cc = np.arange(C_IN)[None, None, :]
        ncol = np.arange(FN)[:, None, None]
        cols = off + ncol * o * o + py[None, :, :, 0] * o + px[None, :, :, 0]
        for dy in range(k):
            for dx in range(k):
                rows = (py + dy) * 64 + (px + dx) * 8 + cc
                wbig[rows[None, :, :, :], cols[:, :, :, None]] = \
                    w[:, :, dy, dx][:, None, None, :]
        cbias[off + np.arange(FN * o * o)] = np.repeat(cb, o * o)
        off += FN * o * o
    return wbig, cbias


def _prep_inputs(inputs):
    x = np.asarray(inputs["x"], np.float32)
    W = np.asarray(inputs["W"], np.float32)
    lora_A = np.asarray(inputs["lora_A"], np.float32)
    lora_B = np.asarray(inputs["lora_B"], np.float32)
    ip_w = np.asarray(inputs["ip_w"], np.float32)
    ip_b = np.asarray(inputs["ip_b"], np.float32)
    out_w = np.asarray(inputs["out_w"], np.float32)
    out_b = np.asarray(inputs["out_b"], np.float32)

    wbig, cbias = _assemble_wbig(inputs)
    ipw_pad = np.zeros((CNN_PAD, SEN), np.float32)
    ipw_pad[:CNN_OUT] = ip_w
    oww_pad = np.zeros((OUT, NUM_PAD), np.float32)
    oww_pad[:, :NUM_OUT] = out_w
    ob_pad = np.zeros(NUM_PAD, np.float32)
    ob_pad[:NUM_OUT] = out_b

    mask = (W != 0).astype(np.float32)
    W1 = W + LORA_SCALE * (lora_A @ lora_B) * mask + np.eye(TOT, dtype=np.float32)

    def bf(a):
        return np.ascontiguousarray(a).astype(BF16)

    shared = {
        "wbig": bf(wbig), "cbias": np.ascontiguousarray(cbias),
        "ipw": bf(ipw_pad), "ipb": np.ascontiguousarray(ip_b),
        "w1r": bf(W1[:KT_RES * 128]), "w1s": bf(W1[KT_RES * 128:]),
        "oww": bf(oww_pad),
    }
    in_maps = []
    for c in range(N_CORES):
        m = dict(shared)
        m["xT"] = bf(x[c * BSH:(c + 1) * BSH].reshape(BSH, 512).T)
        in_maps.append(m)
    return in_maps


def run_on_hw(in_maps, reps: int = 1):
    nc = get_program(reps)
    return run_bass_kernel_spmd(nc, in_maps, list(range(N_CORES)), trace=False)


def kernel(**inputs) -> np.ndarray:
    in_maps = _prep_inputs(inputs)
    res = run_on_hw(in_maps, reps=1)
    out = np.zeros((B, NUM_OUT), np.float32)
    for c in range(N_CORES):
        o = np.asarray(res.results[c]["outT"], np.float32)  # [128, OMT, BSH]
        o = o.transpose(1, 0, 2).reshape(NUM_PAD, BSH)      # row j = m*128+p
        out[c * BSH:(c + 1) * BSH, :] = o[:NUM_OUT].T
    out += np.asarray(inputs["out_b"], np.float32)[None, :]
    return np.ascontiguousarray(out)


# revision 36
# speedup vs baseline: 1.0258x; 1.0258x over previous
"""Trainium2 Bass kernel for nn_BasicCNN (conv bank + LoRA-masked recurrent net).

Pure data-parallel over batch (128 rows/core on 8 cores), ZERO collectives —
under the conservative collective cost model (15us + bytes/40GBps) any
TP-style per-timestep AllGather dwarfs the actual compute, so each core runs
the full network on its batch shard instead:

 - W1 = W + 2*(A@B)*mask + I is built on HOST (fp32) and shipped bf16.
   The +I fold implements the residual. Rows 0:KT_RES*128 are SBUF-resident:
   rows 0:SEN stream INTO the resident tile in t1-consumption order (t1's
   stream IS the resident load, nothing shipped twice); the remaining k-slabs
   are streamed per timestep as [128, 1024] chunks (one per (m-group, k))
   round-robin over the three DMA queues (SP/Pool/Act), ring-buffered so DMA
   hides under the PE matmuls.
 - State kept transposed k-major ([state_dim, batch] in 32 slabs of
   [128, 128]) so W tiles are the stationary operand and the matmul output
   [m-part, batch] is directly the next state slab — no transposes ever.
 - PSUM allows only 8 bank-aligned accumulators, so each timestep runs 4
   m-groups x 8 banks; all relu/bias drains live on DVE (plus Act for the
   tail copies) so no DMA queue mixes in compute.
 - t1 contracts only the sensory block (state1 is zero past SEN); t4 runs as
   two 4-bank column halves with out-proj group 0's contraction split so PE
   never waits on a full drain barrier; out-proj stationary is SBUF-resident
   (loaded during t2 slack); output bias is added on the host.
 - conv bank = one dense [512, 3328] matmul vs a host-assembled scatter of
   the conv kernels, loaded as 16 column-chunks so conv starts ~2.5us in.
"""
import sys

for _p in ("/opt/trn_rl_repo", "/root/.axon_site/_ro/trn_rl_repo"):
    if _p not in sys.path:
        sys.path.append(_p)

import numpy as np
import ml_dtypes

import concourse.bacc as bacc
import concourse.mybir as mybir
import concourse.tile as tile
from concourse.bass_utils import run_bass_kernel_spmd

dt = mybir.dt
BF16 = ml_dtypes.bfloat16
AF = mybir.ActivationFunctionType

N_CORES = 8
B = 1024
HW = 8
C_IN = 8
FN = 16
SEN, INT, OUT = 1024, 2048, 1024
TOT = 4096
CNN_OUT = 3264
CNN_PAD = 3328
NUM_OUT = 1968
NUM_PAD = 2048
LORA_SCALE = 2.0

BSH = B // N_CORES           # 128 batch rows per core
KT = TOT // 128              # 32 k-slabs of state/W
KT_RES = 10                  # W1 k-slabs resident in SBUF
KT_STR = KT - KT_RES         # 19 streamed per full timestep
NG = 4                       # m-groups per timestep
GM = 8                       # m-tiles per group (= PSUM banks)
CONV_MT = CNN_PAD // 128     # 26
SEN_KT = SEN // 128          # 8
OUT_KT = OUT // 128          # 8 (O-block k-slabs for out proj)
OMT = NUM_PAD // 128         # 16


def _build_program(reps: int = 1, use_cc: bool = True):
    nc = bacc.Bacc("TRN2", target_bir_lowering=False, debug=False,
                   enable_asserts=True, num_devices=N_CORES)

    xT_d = nc.dram_tensor("xT", [512, BSH], dt.bfloat16, kind="ExternalInput")
    wbig_d = nc.dram_tensor("wbig", [512, CNN_PAD], dt.bfloat16, kind="ExternalInput")
    cbias_d = nc.dram_tensor("cbias", [CNN_PAD], dt.float32, kind="ExternalInput")
    ipw_d = nc.dram_tensor("ipw", [CNN_PAD, SEN], dt.bfloat16, kind="ExternalInput")
    ipb_d = nc.dram_tensor("ipb", [SEN], dt.float32, kind="ExternalInput")
    w1r_d = nc.dram_tensor("w1r", [KT_RES * 128, TOT], dt.bfloat16, kind="ExternalInput")
    w1s_d = nc.dram_tensor("w1s", [KT_STR * 128, TOT], dt.bfloat16, kind="ExternalInput")
    oww_d = nc.dram_tensor("oww", [OUT, NUM_PAD], dt.bfloat16, kind="ExternalInput")

    # partition-major output layout [p, m, b]; host maps row j = m*128+p
    outT_d = nc.dram_tensor("outT", [128, OMT, BSH], dt.float32, kind="ExternalOutput")

    with tile.TileContext(nc) as tc:
        with tc.tile_pool(name="pers", bufs=1) as pers, \
             tc.tile_pool(name="psum", bufs=8, space="PSUM") as psp, \
             tc.tile_pool(name="wbigp", bufs=4) as wbp, \
             tc.tile_pool(name="ipwp", bufs=10) as ipp, \
             tc.tile_pool(name="wchk", bufs=8) as wcp, \
             tc.tile_pool(name="outp", bufs=1) as otp:

            wres = pers.tile([128, KT_RES, TOT], dt.bfloat16, tag="wres")
            st_a = pers.tile([128, KT, BSH], dt.bfloat16, tag="st_a")
            st_b = pers.tile([128, KT, BSH], dt.bfloat16, tag="st_b")
            ostate = pers.tile([128, OUT_KT, BSH], dt.bfloat16, tag="ostate")
            featT = pers.tile([128, CONV_MT, BSH], dt.bfloat16, tag="featT")
            oww_sb = pers.tile([128, OUT_KT, NUM_PAD], dt.bfloat16, tag="oww_sb")
            xT_sb = pers.tile([128, 4, BSH], dt.bfloat16, tag="xT_sb")
            cbias_sb = pers.tile([128, CONV_MT], dt.float32, tag="cbias_sb")
            ipb_sb = pers.tile([128, SEN_KT], dt.float32, tag="ipb_sb")

            # unified round-robin over the three DMA-capable queues; all
            # elementwise drains live on DVE so no queue mixes DMA + compute
            dmaq = [nc.gpsimd, nc.sync, nc.scalar]
            qctr = [0]

            def dma(out, in_):
                dmaq[qctr[0] % 3].dma_start(out=out, in_=in_)
                qctr[0] += 1

            warm_in = pers.tile([128, 1], dt.float32, tag="warm_in")
            warm_out = pers.tile([128, 1], dt.float32, tag="warm_out")
            nc.gpsimd.memset(warm_in[:], 0.0)

            nc.gpsimd.dma_start(out=xT_sb[:, :, :],
                                in_=xT_d.rearrange("(k p) b -> p k b", p=128))
            nc.sync.dma_start(out=cbias_sb[:], in_=cbias_d.rearrange("(m p) -> p m", p=128))
            nc.sync.dma_start(out=ipb_sb[:], in_=ipb_d.rearrange("(m p) -> p m", p=128))

            for rep in range(reps):
                # ---- conv bank: one dense matmul vs scattered conv kernels ----
                # wbig loaded as 16 column-chunks round-robin over all three
                # DMA queues so conv's first m-tiles start ~2.5us in
                wbig_t = [wbp.tile([128, CNN_PAD], dt.bfloat16, tag="wbig",
                                   name=f"wbig{rep}_{i}") for i in range(4)]
                col_waves = [(0, 256), (256, 832), (832, 1664),
                             (1664, 2496), (2496, CNN_PAD)]
                for c0, c1 in col_waves:
                    for kk in range(4):
                        dma(wbig_t[kk][:, c0:c1], wbig_d[kk * 128:(kk + 1) * 128, c0:c1])
                for m in range(CONV_MT):
                    ps = psp.tile([128, BSH], dt.float32, tag="ps", name=f"cv{rep}_{m}")
                    for kk in range(4):
                        nc.tensor.matmul(ps[:], wbig_t[kk][:, m * 128:(m + 1) * 128],
                                         xT_sb[:, kk, :], start=(kk == 0), stop=(kk == 3))
                    nc.vector.tensor_scalar(featT[:, m, :], ps[:], cbias_sb[:, m:m + 1],
                                            0.0, op0=mybir.AluOpType.add,
                                            op1=mybir.AluOpType.max)

                # ---- input proj -> state1 (k-outer over 26 ipw slabs) ----
                ip_ps = [psp.tile([128, BSH], dt.float32, tag="ps", name=f"ip{rep}_{m}")
                         for m in range(SEN_KT)]
                for k in range(CONV_MT):
                    t = ipp.tile([128, SEN], dt.bfloat16, tag="ipw")
                    dma(t[:], ipw_d[k * 128:(k + 1) * 128, :])
                    for m in range(SEN_KT):
                        nc.tensor.matmul(ip_ps[m][:], t[:, m * 128:(m + 1) * 128],
                                         featT[:, k, :], start=(k == 0),
                                         stop=(k == CONV_MT - 1))
                for m in range(SEN_KT):
                    nc.vector.tensor_scalar(st_a[:, m, :], ip_ps[m][:],
                                            ipb_sb[:, m:m + 1], 0.0,
                                            op0=mybir.AluOpType.add,
                                            op1=mybir.AluOpType.max)

                # ---- t1 (contracts only the SEN block, k0..8): the
                # [128, 1024] pieces stream straight INTO the resident wres
                # tile in consumption order — t1's stream IS the resident
                # load for rows 0:SEN, nothing is shipped twice ----
                cur, nxt = st_a, st_b
                for mg in range(NG):
                    ps = [psp.tile([128, BSH], dt.float32, tag="ps",
                                   name=f"t1r{rep}g{mg}_{i}") for i in range(GM)]
                    for k in range(SEN_KT):
                        if rep == 0:
                            dma(wres[:, k, mg * 1024:(mg + 1) * 1024],
                                w1r_d[k * 128:(k + 1) * 128,
                                      mg * 1024:(mg + 1) * 1024])
                        for m8 in range(GM):
                            m = mg * GM + m8
                            nc.tensor.matmul(ps[m8][:],
                                             wres[:, k, m * 128:(m + 1) * 128],
                                             cur[:, k, :],
                                             start=(k == 0), stop=(k == SEN_KT - 1))
                    for m8 in range(GM):
                        nc.vector.tensor_scalar_max(nxt[:, mg * GM + m8, :],
                                                    ps[m8][:], 0.0)
                    if rep == 0:
                        # the two k-slabs past SEN land behind each group
                        for kr in range(SEN_KT, KT_RES):
                            dma(wres[:, kr, mg * 1024:(mg + 1) * 1024],
                                w1r_d[kr * 128:(kr + 1) * 128,
                                      mg * 1024:(mg + 1) * 1024])
                cur, nxt = nxt, cur

                # warm the Act engine's activation table (Copy) off the
                # critical path so the tail bias-adds pay no table load
                if rep == 0:
                    nc.scalar.activation(warm_out[:], warm_in[:], AF.Copy, bias=0.0)

                # ---- recurrence t2..t3 (full contraction) ----
                for t in (2, 3):
                    for mg in range(NG):
                        if rep == 0 and t == 2:
                            # out-proj stationary resident load, slipped into
                            # t2's DMA slack (2 k-slabs per group)
                            dma(oww_sb[:, 2 * mg:2 * mg + 2, :],
                                oww_d[256 * mg:256 * (mg + 1), :]
                                .rearrange("(k p) o -> p k o", p=128))
                        ps = [psp.tile([128, BSH], dt.float32, tag="ps",
                                       name=f"t{t}r{rep}g{mg}_{i}") for i in range(GM)]
                        chunk = None
                        for k in range(KT):
                            if k >= KT_RES:
                                chunk = wcp.tile([128, 1024], dt.bfloat16, tag="wchk")
                                dma(chunk[:],
                                    w1s_d[(k - KT_RES) * 128:(k - KT_RES + 1) * 128,
                                          mg * 1024:(mg + 1) * 1024])
                            for m8 in range(GM):
                                m = mg * GM + m8
                                if k < KT_RES:
                                    wap = wres[:, k, m * 128:(m + 1) * 128]
                                else:
                                    wap = chunk[:, m8 * 128:(m8 + 1) * 128]
                                nc.tensor.matmul(ps[m8][:], wap, cur[:, k, :],
                                                 start=(k == 0), stop=(k == KT - 1))
                        for m8 in range(GM):
                            nc.vector.tensor_scalar_max(nxt[:, mg * GM + m8, :],
                                                        ps[m8][:], 0.0)
                    cur, nxt = nxt, cur

                # ---- t4 (O block only) in two 4-bank column halves, then
                # out-proj in four 4-bank groups with the first group's
                # contraction split so PE never stalls on a full drain
                # barrier; output bias is added on the host ----
                ostg = otp.tile([128, OMT, BSH], dt.float32, tag="ostg")

                t4_ps = {}
                for half in range(2):
                    for m4 in range(4):
                        t4_ps[(half, m4)] = psp.tile(
                            [128, BSH], dt.float32, tag="ps",
                            name=f"t4r{rep}h{half}_{m4}")
                    if half == 0:
                        op_g0 = [psp.tile([128, BSH], dt.float32, tag="ps",
                                          name=f"opr{rep}g0_{i}") for i in range(4)]
                    chunk = None
                    for k in range(KT):
                        if k >= KT_RES:
                            chunk = wcp.tile([128, 512], dt.bfloat16, tag="wchk",
                                             name=f"t4ch{rep}h{half}_{k}")
                            dma(chunk[:],
                                w1s_d[(k - KT_RES) * 128:(k - KT_RES + 1) * 128,
                                      3072 + half * 512:3072 + (half + 1) * 512])
                        for m4 in range(4):
                            c0 = 3072 + half * 512 + m4 * 128
                            if k < KT_RES:
                                wap = wres[:, k, c0:c0 + 128]
                            else:
                                wap = chunk[:, m4 * 128:(m4 + 1) * 128]
                            nc.tensor.matmul(t4_ps[(half, m4)][:], wap, cur[:, k, :],
                                             start=(k == 0), stop=(k == KT - 1))
                    for m4 in range(4):
                        sl = half * 4 + m4
                        if m4 % 2 == 0:
                            nc.vector.tensor_scalar_max(ostate[:, sl, :],
                                                        t4_ps[(half, m4)][:], 0.0)
                        else:
                            nc.scalar.activation(ostate[:, sl, :],
                                                 t4_ps[(half, m4)][:], AF.Relu)
                    if half == 0:
                        # partial out-proj group 0 over the first four ostate
                        # slabs while t4's second half still computes
                        for k in range(4):
                            for j4 in range(4):
                                nc.tensor.matmul(op_g0[j4][:],
                                                 oww_sb[:, k, j4 * 128:(j4 + 1) * 128],
                                                 ostate[:, k, :],
                                                 start=(k == 0), stop=False)

                def opj_drain(g, ps4):
                    for j4 in range(4):
                        jm = g * 4 + j4
                        if j4 % 2 == 0:
                            nc.vector.tensor_scalar_add(ostg[:, jm, :], ps4[j4][:], 0.0)
                        else:
                            nc.scalar.activation(ostg[:, jm, :], ps4[j4][:],
                                                 AF.Copy, bias=0.0)
                        if g == 3 and j4 == 1:
                            dma(outT_d[:, 12:14, :], ostg[:, 12:14, :])
                    if g == 3:
                        dma(outT_d[:, 14:16, :], ostg[:, 14:16, :])
                    else:
                        dma(outT_d[:, g * 4:(g + 1) * 4, :],
                            ostg[:, g * 4:(g + 1) * 4, :])

                # finish group 0 (k4..8) and run groups 1..3
                for k in range(4, OUT_KT):
                    for j4 in range(4):
                        nc.tensor.matmul(op_g0[j4][:],
                                         oww_sb[:, k, j4 * 128:(j4 + 1) * 128],
                                         ostate[:, k, :], start=False,
                                         stop=(k == OUT_KT - 1))
                opj_drain(0, op_g0)
                for g in range(1, 4):
                    psg = [psp.tile([128, BSH], dt.float32, tag="ps",
                                    name=f"opr{rep}g{g}_{i}") for i in range(4)]
                    for k in range(OUT_KT):
                        for j4 in range(4):
                            nc.tensor.matmul(psg[j4][:],
                                             oww_sb[:, k,
                                                    g * 512 + j4 * 128:g * 512 + (j4 + 1) * 128],
                                             ostate[:, k, :], start=(k == 0),
                                             stop=(k == OUT_KT - 1))
                    opj_drain(g, psg)

    nc.compile()
    return nc


_PROGRAM_CACHE: dict = {}


def get_program(reps: int = 1, use_cc: bool = True):
    key = (reps, use_cc)
    if key not in _PROGRAM_CACHE:
        _PROGRAM_CACHE[key] = _build_program(reps, use_cc)
    return _PROGRAM_CACHE[key]


def _assemble_wbig(inputs):
    wbig = np.zeros((512, CNN_PAD), np.float32)
    cbias = np.zeros(CNN_PAD, np.float32)
    off = 0
    for k in range(1, 9):
        o = HW - k + 1
        w = np.asarray(inputs[f"conv_w{k}"], np.float32)
        cb = np.asarray(inputs["conv_b"], np.float32)[k - 1]
        py = np.arange(o)[:, None, None]
        px = np.arange(o)[None, :, None]
        cc = np.arange(C_IN)[None, None, :]
        ncol = np.arange(FN)[:, None, None]
        cols = off + ncol * o * o + py[None, :, :, 0] * o + px[None, :, :, 0]
        for dy in range(k):
            for dx in range(k):
                rows = (py + dy) * 64 + (px + dx) * 8 + cc
                wbig[rows[None, :, :, :], cols[:, :, :, None]] = \
                    w[:, :, dy, dx][:, None, None, :]
        cbias[off + np.arange(FN * o * o)] = np.repeat(cb, o * o)
        off += FN * o * o
    return wbig, cbias


def _prep_inputs(inputs):
    x = np.asarray(inputs["x"], np.float32)
    W = np.asarray(inputs["W"], np.float32)
    lora_A = np.asarray(inputs["lora_A"], np.float32)
    lora_B = np.asarray(inputs["lora_B"], np.float32)
    ip_w = np.asarray(inputs["ip_w"], np.float32)
    ip_b = np.asarray(inputs["ip_b"], np.float32)
    out_w = np.asarray(inputs["out_w"], np.float32)
    out_b = np.asarray(inputs["out_b"], np.float32)

    wbig, cbias = _assemble_wbig(inputs)
    ipw_pad = np.zeros((CNN_PAD, SEN), np.float32)
    ipw_pad[:CNN_OUT] = ip_w
    oww_pad = np.zeros((OUT, NUM_PAD), np.float32)
    oww_pad[:, :NUM_OUT] = out_w
    ob_pad = np.zeros(NUM_PAD, np.float32)
    ob_pad[:NUM_OUT] = out_b

    mask = (W != 0).astype(np.float32)
    W1 = W + LORA_SCALE * (lora_A @ lora_B) * mask + np.eye(TOT, dtype=np.float32)

    def bf(a):
        return np.ascontiguousarray(a).astype(BF16)

    shared = {
        "wbig": bf(wbig), "cbias": np.ascontiguousarray(cbias),
        "ipw": bf(ipw_pad), "ipb": np.ascontiguousarray(ip_b),
        "w1r": bf(W1[:KT_RES * 128]), "w1s": bf(W1[KT_RES * 128:]),
        "oww": bf(oww_pad),
    }
    in_maps = []
    for c in range(N_CORES):
        m = dict(shared)
        m["xT"] = bf(x[c * BSH:(c + 1) * BSH].reshape(BSH, 512).T)
        in_maps.append(m)
    return in_maps


def run_on_hw(in_maps, reps: int = 1):
    nc = get_program(reps)
    return run_bass_kernel_spmd(nc, in_maps, list(range(N_CORES)), trace=False)


def kernel(**inputs) -> np.ndarray:
    in_maps = _prep_inputs(inputs)
    res = run_on_hw(in_maps, reps=1)
    out = np.zeros((B, NUM_OUT), np.float32)
    for c in range(N_CORES):
        o = np.asarray(res.results[c]["outT"], np.float32)  # [128, OMT, BSH]
        o = o.transpose(1, 0, 2).reshape(NUM_PAD, BSH)      # row j = m*128+p
        out[c * BSH:(c + 1) * BSH, :] = o[:NUM_OUT].T
    out += np.asarray(inputs["out_b"], np.float32)[None, :]
    return np.ascontiguousarray(out)


# revision 37
# speedup vs baseline: 1.0270x; 1.0011x over previous
"""Trainium2 Bass kernel for nn_BasicCNN (conv bank + LoRA-masked recurrent net).

Pure data-parallel over batch (128 rows/core on 8 cores), ZERO collectives —
under the conservative collective cost model (15us + bytes/40GBps) any
TP-style per-timestep AllGather dwarfs the actual compute, so each core runs
the full network on its batch shard instead:

 - W1 = W + 2*(A@B)*mask + I is built on HOST (fp32) and shipped bf16.
   The +I fold implements the residual. Rows 0:KT_RES*128 are SBUF-resident:
   rows 0:SEN stream INTO the resident tile in t1-consumption order (t1's
   stream IS the resident load, nothing shipped twice); the remaining k-slabs
   are streamed per timestep as [128, 1024] chunks (one per (m-group, k))
   round-robin over the three DMA queues (SP/Pool/Act), ring-buffered so DMA
   hides under the PE matmuls.
 - State kept transposed k-major ([state_dim, batch] in 32 slabs of
   [128, 128]) so W tiles are the stationary operand and the matmul output
   [m-part, batch] is directly the next state slab — no transposes ever.
 - PSUM allows only 8 bank-aligned accumulators, so each timestep runs 4
   m-groups x 8 banks; all relu/bias drains live on DVE (plus Act for the
   tail copies) so no DMA queue mixes in compute.
 - t1 contracts only the sensory block (state1 is zero past SEN); t4 runs as
   two 4-bank column halves with out-proj group 0's contraction split so PE
   never waits on a full drain barrier; out-proj stationary is SBUF-resident
   (loaded during t2 slack); output bias is added on the host.
 - conv bank = one dense [512, 3328] matmul vs a host-assembled scatter of
   the conv kernels, loaded as 16 column-chunks so conv starts ~2.5us in.
"""
import sys

for _p in ("/opt/trn_rl_repo", "/root/.axon_site/_ro/trn_rl_repo"):
    if _p not in sys.path:
        sys.path.append(_p)

import numpy as np
import ml_dtypes

import concourse.bacc as bacc
import concourse.mybir as mybir
import concourse.tile as tile
from concourse.bass_utils import run_bass_kernel_spmd

dt = mybir.dt
BF16 = ml_dtypes.bfloat16
AF = mybir.ActivationFunctionType

N_CORES = 8
B = 1024
HW = 8
C_IN = 8
FN = 16
SEN, INT, OUT = 1024, 2048, 1024
TOT = 4096
CNN_OUT = 3264
CNN_PAD = 3328
NUM_OUT = 1968
NUM_PAD = 2048
LORA_SCALE = 2.0

BSH = B // N_CORES           # 128 batch rows per core
KT = TOT // 128              # 32 k-slabs of state/W
KT_RES = 10                  # W1 k-slabs resident in SBUF
KT_STR = KT - KT_RES         # 19 streamed per full timestep
NG = 4                       # m-groups per timestep
GM = 8                       # m-tiles per group (= PSUM banks)
CONV_MT = CNN_PAD // 128     # 26
SEN_KT = SEN // 128          # 8
OUT_KT = OUT // 128          # 8 (O-block k-slabs for out proj)
OMT = NUM_PAD // 128         # 16


def _build_program(reps: int = 1, use_cc: bool = True):
    nc = bacc.Bacc("TRN2", target_bir_lowering=False, debug=False,
                   enable_asserts=True, num_devices=N_CORES)

    xT_d = nc.dram_tensor("xT", [512, BSH], dt.bfloat16, kind="ExternalInput")
    wbig_d = nc.dram_tensor("wbig", [512, CNN_PAD], dt.bfloat16, kind="ExternalInput")
    cbias_d = nc.dram_tensor("cbias", [CNN_PAD], dt.float32, kind="ExternalInput")
    ipw_d = nc.dram_tensor("ipw", [CNN_PAD, SEN], dt.bfloat16, kind="ExternalInput")
    ipb_d = nc.dram_tensor("ipb", [SEN], dt.float32, kind="ExternalInput")
    w1r_d = nc.dram_tensor("w1r", [KT_RES * 128, TOT], dt.bfloat16, kind="ExternalInput")
    w1s_d = nc.dram_tensor("w1s", [KT_STR * 128, TOT], dt.bfloat16, kind="ExternalInput")
    oww_d = nc.dram_tensor("oww", [OUT, NUM_PAD], dt.bfloat16, kind="ExternalInput")

    # partition-major output layout [p, m, b]; host maps row j = m*128+p
    outT_d = nc.dram_tensor("outT", [128, OMT, BSH], dt.float32, kind="ExternalOutput")

    with tile.TileContext(nc) as tc:
        with tc.tile_pool(name="pers", bufs=1) as pers, \
             tc.tile_pool(name="psum", bufs=8, space="PSUM") as psp, \
             tc.tile_pool(name="wbigp", bufs=4) as wbp, \
             tc.tile_pool(name="ipwp", bufs=10) as ipp, \
             tc.tile_pool(name="wchk", bufs=8) as wcp, \
             tc.tile_pool(name="outp", bufs=1) as otp:

            wres = pers.tile([128, KT_RES, TOT], dt.bfloat16, tag="wres")
            st_a = pers.tile([128, KT, BSH], dt.bfloat16, tag="st_a")
            st_b = pers.tile([128, KT, BSH], dt.bfloat16, tag="st_b")
            ostate = pers.tile([128, OUT_KT, BSH], dt.bfloat16, tag="ostate")
            featT = pers.tile([128, CONV_MT, BSH], dt.bfloat16, tag="featT")
            oww_sb = pers.tile([128, OUT_KT, NUM_PAD], dt.bfloat16, tag="oww_sb")
            xT_sb = pers.tile([128, 4, BSH], dt.bfloat16, tag="xT_sb")
            cbias_sb = pers.tile([128, CONV_MT], dt.float32, tag="cbias_sb")
            ipb_sb = pers.tile([128, SEN_KT], dt.float32, tag="ipb_sb")

            # unified round-robin over the three DMA-capable queues; all
            # elementwise drains live on DVE so no queue mixes DMA + compute
            dmaq = [nc.gpsimd, nc.sync, nc.scalar]
            qctr = [0]

            def dma(out, in_):
                dmaq[qctr[0] % 3].dma_start(out=out, in_=in_)
                qctr[0] += 1

            warm_in = pers.tile([128, 1], dt.float32, tag="warm_in")
            warm_out = pers.tile([128, 1], dt.float32, tag="warm_out")

            nc.gpsimd.dma_start(out=xT_sb[:, :, :],
                                in_=xT_d.rearrange("(k p) b -> p k b", p=128))
            nc.gpsimd.memset(warm_in[:], 0.0)
            nc.sync.dma_start(out=cbias_sb[:], in_=cbias_d.rearrange("(m p) -> p m", p=128))
            nc.sync.dma_start(out=ipb_sb[:], in_=ipb_d.rearrange("(m p) -> p m", p=128))

            for rep in range(reps):
                # ---- conv bank: one dense matmul vs scattered conv kernels ----
                # wbig loaded as 16 column-chunks round-robin over all three
                # DMA queues so conv's first m-tiles start ~2.5us in
                wbig_t = [wbp.tile([128, CNN_PAD], dt.bfloat16, tag="wbig",
                                   name=f"wbig{rep}_{i}") for i in range(4)]
                col_waves = [(0, 256), (256, 832), (832, 1664),
                             (1664, 2496), (2496, CNN_PAD)]
                for c0, c1 in col_waves:
                    for kk in range(4):
                        dma(wbig_t[kk][:, c0:c1], wbig_d[kk * 128:(kk + 1) * 128, c0:c1])
                for m in range(CONV_MT):
                    ps = psp.tile([128, BSH], dt.float32, tag="ps", name=f"cv{rep}_{m}")
                    for kk in range(4):
                        nc.tensor.matmul(ps[:], wbig_t[kk][:, m * 128:(m + 1) * 128],
                                         xT_sb[:, kk, :], start=(kk == 0), stop=(kk == 3))
                    nc.vector.tensor_scalar(featT[:, m, :], ps[:], cbias_sb[:, m:m + 1],
                                            0.0, op0=mybir.AluOpType.add,
                                            op1=mybir.AluOpType.max)

                # ---- input proj -> state1 (k-outer over 26 ipw slabs) ----
                ip_ps = [psp.tile([128, BSH], dt.float32, tag="ps", name=f"ip{rep}_{m}")
                         for m in range(SEN_KT)]
                for k in range(CONV_MT):
                    t = ipp.tile([128, SEN], dt.bfloat16, tag="ipw")
                    dma(t[:], ipw_d[k * 128:(k + 1) * 128, :])
                    for m in range(SEN_KT):
                        nc.tensor.matmul(ip_ps[m][:], t[:, m * 128:(m + 1) * 128],
                                         featT[:, k, :], start=(k == 0),
                                         stop=(k == CONV_MT - 1))
                for m in range(SEN_KT):
                    nc.vector.tensor_scalar(st_a[:, m, :], ip_ps[m][:],
                                            ipb_sb[:, m:m + 1], 0.0,
                                            op0=mybir.AluOpType.add,
                                            op1=mybir.AluOpType.max)

                # ---- t1 (contracts only the SEN block, k0..8): the
                # [128, 1024] pieces stream straight INTO the resident wres
                # tile in consumption order — t1's stream IS the resident
                # load for rows 0:SEN, nothing is shipped twice ----
                cur, nxt = st_a, st_b
                for mg in range(NG):
                    ps = [psp.tile([128, BSH], dt.float32, tag="ps",
                                   name=f"t1r{rep}g{mg}_{i}") for i in range(GM)]
                    for k in range(SEN_KT):
                        if rep == 0:
                            dma(wres[:, k, mg * 1024:(mg + 1) * 1024],
                                w1r_d[k * 128:(k + 1) * 128,
                                      mg * 1024:(mg + 1) * 1024])
                        for m8 in range(GM):
                            m = mg * GM + m8
                            nc.tensor.matmul(ps[m8][:],
                                             wres[:, k, m * 128:(m + 1) * 128],
                                             cur[:, k, :],
                                             start=(k == 0), stop=(k == SEN_KT - 1))
                    for m8 in range(GM):
                        nc.vector.tensor_scalar_max(nxt[:, mg * GM + m8, :],
                                                    ps[m8][:], 0.0)
                    if rep == 0:
                        # the two k-slabs past SEN land behind each group
                        for kr in range(SEN_KT, KT_RES):
                            dma(wres[:, kr, mg * 1024:(mg + 1) * 1024],
                                w1r_d[kr * 128:(kr + 1) * 128,
                                      mg * 1024:(mg + 1) * 1024])
                cur, nxt = nxt, cur

                # warm the Act engine's activation table (Copy) off the
                # critical path so the tail bias-adds pay no table load
                if rep == 0:
                    nc.scalar.activation(warm_out[:], warm_in[:], AF.Copy, bias=0.0)

                # ---- recurrence t2..t3 (full contraction) ----
                for t in (2, 3):
                    for mg in range(NG):
                        if rep == 0 and t == 2:
                            # out-proj stationary resident load, slipped into
                            # t2's DMA slack (2 k-slabs per group)
                            dma(oww_sb[:, 2 * mg:2 * mg + 2, :],
                                oww_d[256 * mg:256 * (mg + 1), :]
                                .rearrange("(k p) o -> p k o", p=128))
                        ps = [psp.tile([128, BSH], dt.float32, tag="ps",
                                       name=f"t{t}r{rep}g{mg}_{i}") for i in range(GM)]
                        chunk = None
                        for k in range(KT):
                            if k >= KT_RES:
                                chunk = wcp.tile([128, 1024], dt.bfloat16, tag="wchk")
                                dma(chunk[:],
                                    w1s_d[(k - KT_RES) * 128:(k - KT_RES + 1) * 128,
                                          mg * 1024:(mg + 1) * 1024])
                            for m8 in range(GM):
                                m = mg * GM + m8
                                if k < KT_RES:
                                    wap = wres[:, k, m * 128:(m + 1) * 128]
                                else:
                                    wap = chunk[:, m8 * 128:(m8 + 1) * 128]
                                nc.tensor.matmul(ps[m8][:], wap, cur[:, k, :],
                                                 start=(k == 0), stop=(k == KT - 1))
                        for m8 in range(GM):
                            nc.vector.tensor_scalar_max(nxt[:, mg * GM + m8, :],
                                                        ps[m8][:], 0.0)
                    cur, nxt = nxt, cur

                # ---- t4 (O block only) in two 4-bank column halves, then
                # out-proj in four 4-bank groups with the first group's
                # contraction split so PE never stalls on a full drain
                # barrier; output bias is added on the host ----
                ostg = otp.tile([128, OMT, BSH], dt.float32, tag="ostg")

                t4_ps = {}
                for half in range(2):
                    for m4 in range(4):
                        t4_ps[(half, m4)] = psp.tile(
                            [128, BSH], dt.float32, tag="ps",
                            name=f"t4r{rep}h{half}_{m4}")
                    if half == 0:
                        op_g0 = [psp.tile([128, BSH], dt.float32, tag="ps",
                                          name=f"opr{rep}g0_{i}") for i in range(4)]
                    chunk = None
                    for k in range(KT):
                        if k >= KT_RES:
                            chunk = wcp.tile([128, 512], dt.bfloat16, tag="wchk",
                                             name=f"t4ch{rep}h{half}_{k}")
                            dma(chunk[:],
                                w1s_d[(k - KT_RES) * 128:(k - KT_RES + 1) * 128,
                                      3072 + half * 512:3072 + (half + 1) * 512])
                        for m4 in range(4):
                            c0 = 3072 + half * 512 + m4 * 128
                            if k < KT_RES:
                                wap = wres[:, k, c0:c0 + 128]
                            else:
                                wap = chunk[:, m4 * 128:(m4 + 1) * 128]
                            nc.tensor.matmul(t4_ps[(half, m4)][:], wap, cur[:, k, :],
                                             start=(k == 0), stop=(k == KT - 1))
                    for m4 in range(4):
                        sl = half * 4 + m4
                        if m4 % 2 == 0:
                            nc.vector.tensor_scalar_max(ostate[:, sl, :],
                                                        t4_ps[(half, m4)][:], 0.0)
                        else:
                            nc.scalar.activation(ostate[:, sl, :],
                                                 t4_ps[(half, m4)][:], AF.Relu)
                    if half == 0:
                        # partial out-proj group 0 over the first four ostate
                        # slabs while t4's second half still computes
                        for k in range(4):
                            for j4 in range(4):
                                nc.tensor.matmul(op_g0[j4][:],
                                                 oww_sb[:, k, j4 * 128:(j4 + 1) * 128],
                                                 ostate[:, k, :],
                                                 start=(k == 0), stop=False)

                def opj_drain(g, ps4):
                    for j4 in range(4):
                        jm = g * 4 + j4
                        if j4 % 2 == 0:
                            nc.vector.tensor_scalar_add(ostg[:, jm, :], ps4[j4][:], 0.0)
                        else:
                            nc.scalar.activation(ostg[:, jm, :], ps4[j4][:],
                                                 AF.Copy, bias=0.0)
                    dma(outT_d[:, g * 4:(g + 1) * 4, :], ostg[:, g * 4:(g + 1) * 4, :])

                # finish group 0 (k4..8) and run groups 1..3
                for k in range(4, OUT_KT):
                    for j4 in range(4):
                        nc.tensor.matmul(op_g0[j4][:],
                                         oww_sb[:, k, j4 * 128:(j4 + 1) * 128],
                                         ostate[:, k, :], start=False,
                                         stop=(k == OUT_KT - 1))
                opj_drain(0, op_g0)
                for gw in ((4, 4), (8, 4), (12, 2), (14, 2)):
                    lo, w = gw
                    psg = [psp.tile([128, BSH], dt.float32, tag="ps",
                                    name=f"opr{rep}o{lo}_{i}") for i in range(w)]
                    for k in range(OUT_KT):
                        for j in range(w):
                            c0 = (lo + j) * 128
                            nc.tensor.matmul(psg[j][:],
                                             oww_sb[:, k, c0:c0 + 128],
                                             ostate[:, k, :], start=(k == 0),
                                             stop=(k == OUT_KT - 1))
                    for j in range(w):
                        jm = lo + j
                        if j % 2 == 0:
                            nc.vector.tensor_scalar_add(ostg[:, jm, :], psg[j][:], 0.0)
                        else:
                            nc.scalar.activation(ostg[:, jm, :], psg[j][:],
                                                 AF.Copy, bias=0.0)
                    dma(outT_d[:, lo:lo + w, :], ostg[:, lo:lo + w, :])

    nc.compile()
    return nc


_PROGRAM_CACHE: dict = {}


def get_program(reps: int = 1, use_cc: bool = True):
    key = (reps, use_cc)
    if key not in _PROGRAM_CACHE:
        _PROGRAM_CACHE[key] = _build_program(reps, use_cc)
    return _PROGRAM_CACHE[key]


def _assemble_wbig(inputs):
    wbig = np.zeros((512, CNN_PAD), np.float32)
    cbias = np.zeros(CNN_PAD, np.float32)
    off = 0
    for k in range(1, 9):
        o = HW - k + 1
        w = np.asarray(inputs[f"conv_w{k}"], np.float32)
        cb = np.asarray(inputs["conv_b"], np.float32)[k - 1]
        py = np.arange(o)[:, None, None]
        px = np.arange(o)[None, :, None]
        cc = np.arange(C_IN)[None, None, :]
        ncol = np.arange(FN)[:, None, None]
        cols = off + ncol * o * o + py[None, :, :, 0] * o + px[None, :, :, 0]
        for dy in range(k):
            for dx in range(k):
                rows = (py + dy) * 64 + (px + dx) * 8 + cc
                wbig[rows[None, :, :, :], cols[:, :, :, None]] = \
                    w[:, :, dy, dx][:, None, None, :]
        cbias[off + np.arange(FN * o * o)] = np.repeat(cb, o * o)
        off += FN * o * o
    return wbig, cbias


def _prep_inputs(inputs):
    x = np.asarray(inputs["x"], np.float32)
    W = np.asarray(inputs["W"], np.float32)
    lora_A = np.asarray(inputs["lora_A"], np.float32)
    lora_B = np.asarray(inputs["lora_B"], np.float32)
    ip_w = np.asarray(inputs["ip_w"], np.float32)
    ip_b = np.asarray(inputs["ip_b"], np.float32)
    out_w = np.asarray(inputs["out_w"], np.float32)
    out_b = np.asarray(inputs["out_b"], np.float32)

    wbig, cbias = _assemble_wbig(inputs)
    ipw_pad = np.zeros((CNN_PAD, SEN), np.float32)
    ipw_pad[:CNN_OUT] = ip_w
    oww_pad = np.zeros((OUT, NUM_PAD), np.float32)
    oww_pad[:, :NUM_OUT] = out_w
    ob_pad = np.zeros(NUM_PAD, np.float32)
    ob_pad[:NUM_OUT] = out_b

    mask = (W != 0).astype(np.float32)
    W1 = W + LORA_SCALE * (lora_A @ lora_B) * mask + np.eye(TOT, dtype=np.float32)

    def bf(a):
        return np.ascontiguousarray(a).astype(BF16)

    shared = {
        "wbig": bf(wbig), "cbias": np.ascontiguousarray(cbias),
        "ipw": bf(ipw_pad), "ipb": np.ascontiguousarray(ip_b),
        "w1r": bf(W1[:KT_RES * 128]), "w1s": bf(W1[KT_RES * 128:]),
        "oww": bf(oww_pad),
    }
    in_maps = []
    for c in range(N_CORES):
        m = dict(shared)
        m["xT"] = bf(x[c * BSH:(c + 1) * BSH].reshape(BSH, 512).T)
        in_maps.append(m)
    return in_maps


def run_on_hw(in_maps, reps: int = 1):
    nc = get_program(reps)
    return run_bass_kernel_spmd(nc, in_maps, list(range(N_CORES)), trace=False)


def kernel(**inputs) -> np.ndarray:
    in_maps = _prep_inputs(inputs)
    res = run_on_hw(in_maps, reps=1)
    out = np.zeros((B, NUM_OUT), np.float32)
    for c in range(N_CORES):
        o = np.asarray(res.results[c]["outT"], np.float32)  # [128, OMT, BSH]
        o = o.transpose(1, 0, 2).reshape(NUM_PAD, BSH)      # row j = m*128+p
        out[c * BSH:(c + 1) * BSH, :] = o[:NUM_OUT].T
    out += np.asarray(inputs["out_b"], np.float32)[None, :]
    return np.ascontiguousarray(out)
